# revision 16
# baseline (speedup 1.0000x reference)
"""Trainium2 Bass kernel for nn_ActionDecoder (moe_routing) — fp8 DoubleRow.

Data-parallel across 8 NeuronCores: batch 4096 -> 512 per core, weights
replicated. Host deals samples to cores balanced per command value and sorts
each core's 512 samples by command, so each head's samples occupy a fixed
column segment [a_h, e_h) (identical offsets on all cores -> one SPMD graph).

All GEMMs run in fp8-e4m3 DoubleRow perf mode (2 k-rows per cycle, 157 TF/s)
with f32 PSUM accumulation. Power-of-two scales keep quantization exact to
descale: x as-is, W_fc*32 -> hp stores 32*h' (range <= 178 < 240 fp8 max),
W1*32 with b1 riding an augmented constant row, z1 stores 16*relu(.),
W2*16 -> z2 = psum/256 + b2.

Per core:
  hp  = relu((x@32Wfc) + 32 b_fc)             [128x8 x 512] fp8, 4 pair tiles
  per head h on columns [a_h, e_h):
    z1 = relu(psum/64)                        [256, C] -> [128,2,C] fp8
    z2 = W2'.T z1 / 256 + b2'                 [4, C]
    z2all[:, seg] += z2 * onehot(command)
  out = [5*tanh(mean/5); softplus(std)]       [4, 512] f32
"""

import numpy as np
import ml_dtypes

B = 4096
EMBED = 8192
U0 = 1024
U1 = 256
EGO = 3
H = 6
NCORES = 8
BC = B // NCORES          # 512 batch per core
KP = EMBED // 256         # 32 trunk k-pairs (DoubleRow: 256 k per matmul)
KQ = EMBED // 512         # 16 trunk k-quads (one x DMA each, 2KB lines)
PB = 4                    # trunk pairs per block
NCH = U0 // 128           # 8 trunk n-chunks
MC = U1 // 128            # 2 head m-chunks
NWARM = 64                # junk matmuls to warm the PE clock during DMA ramp
MEAN_SCALE = 5.0
INIT_STD = 5.0
MIN_STD = 1e-4
SH = 32.0                 # scale on W_fc / hp
SW1 = 32.0                # scale on W1 (+bias row)
SZ = 16.0                 # scale on stored z1
SW2 = 16.0                # scale on W2

FP8 = ml_dtypes.float8_e4m3   # TRN fp8_e4m3 semantics (max 240)


def _build_graph(seg):
    """seg: list of (a_h, C_h) column segments per head, identical on all cores."""
    import concourse.mybir as mybir
    import concourse.tile as tile
    from concourse import bacc

    dt = mybir.dt
    AF = mybir.ActivationFunctionType
    DR = mybir.MatmulPerfMode.DoubleRow

    nc = bacc.Bacc("TRN2", target_bir_lowering=False, debug=False)

    xp = nc.dram_tensor("xp", [KQ * 128, 4, BC], dt.float8e4, kind="ExternalInput")
    wp = nc.dram_tensor("wp", [KP * 128, 2, U0], dt.float8e4, kind="ExternalInput")
    bfcT = nc.dram_tensor("bfcT", [128, NCH], dt.float32, kind="ExternalInput")
    egoT = nc.dram_tensor("egoT", [EGO + 1, BC], dt.float8e4, kind="ExternalInput")
    w1pd = nc.dram_tensor("w1pd", [128, H * 4 * 2, 256], dt.float8e4, kind="ExternalInput")
    w1ed = nc.dram_tensor("w1ed", [128, H * 256], dt.float8e4, kind="ExternalInput")
    w2pd = nc.dram_tensor("w2pd", [128, H * 2, 4], dt.float8e4, kind="ExternalInput")
    b2m = nc.dram_tensor("b2m", [4, H], dt.float32, kind="ExternalInput")
    hid = nc.dram_tensor("hid", [4, BC], dt.float32, kind="ExternalInput")
    epi = nc.dram_tensor("epi", [4, 5], dt.float32, kind="ExternalInput")
    out_d = nc.dram_tensor("out", [4, BC], dt.float32, kind="ExternalOutput")

    with tile.TileContext(nc) as tc:
        with (
            tc.tile_pool(name="const", bufs=1) as const,
            tc.tile_pool(name="xk", bufs=KP) as xpool,
            tc.tile_pool(name="wk", bufs=KP) as wpool,
            tc.tile_pool(name="hp", bufs=1) as hpool,
            tc.tile_pool(name="z1", bufs=1) as zpool,
            tc.tile_pool(name="ps", bufs=8, space="PSUM") as psum,
        ):
            # constants / small inputs on the slow sync queue (~2MB total
            # incl. w1): the fast scalar/gpsimd queues carry only x and W.
            bfc_t = const.tile([128, NCH], dt.float32, tag="bfc")
            nc.sync.dma_start(out=bfc_t[:], in_=bfcT[:])
            b2_t = const.tile([4, H], dt.float32, tag="b2")
            nc.sync.dma_start(out=b2_t[:], in_=b2m[:])
            hid_t = const.tile([4, BC], dt.float32, tag="hid")
            nc.sync.dma_start(out=hid_t[:], in_=hid[:])
            w2_t = const.tile([128, H * 2, 4], dt.float8e4, tag="w2")
            nc.sync.dma_start(out=w2_t[:], in_=w2pd[:])
            epi_t = const.tile([4, 5], dt.float32, tag="epi")
            nc.sync.dma_start(out=epi_t[:], in_=epi[:])
            w1p_t = const.tile([128, H * 4 * 2, 256], dt.float8e4, tag="w1p")
            w1e_t = const.tile([128, H * 256], dt.float8e4, tag="w1e")

            # h' ego chunk: rows 0-2 = 32*ego, row 3 = 32 (bias row), rest 0
            hpe = hpool.tile([128, BC], dt.float8e4, tag="hpe")
            nc.vector.memset(hpe[:], 0.0)

            ps_h = [
                psum.tile([128, BC], dt.float32, tag="ps", name=f"ps_h{n}")
                for n in range(NCH)
            ]

            # PE warmup: junk matmuls on the zeroed tile keep the PE activity
            # monitor busy while the first trunk chunks stream in, so the
            # trunk starts at 2.4 GHz instead of 1.2.
            for i in range(NWARM):
                nc.tensor.matmul(
                    ps_h[NCH - 1][:, 0:64],
                    hpe[:, 0:128],
                    hpe[:, 128:192],
                    start=True,
                    stop=True,
                )

            hp_pair = [
                hpool.tile([128, 2, BC], dt.float8e4, tag=f"hpp{t}", name=f"hpp{t}")
                for t in range(4)
            ]

            # trunk: hp = relu((32Wfc).T @ x + 32 b_fc), DoubleRow over
            # k-pairs; x streams in k-quads (one 256KB DMA, 2KB lines).
            qblocks = [1, 1] + [2] * ((KQ - 2) // 2)
            w1_next = 0
            qstart = 0
            for qb, qbsz in enumerate(qblocks):
                last_block = qb == len(qblocks) - 1
                xts, wts = [], []
                for qq_ in range(qbsz):
                    qq = qstart + qq_
                    xk = xpool.tile([128, 4, BC], dt.float8e4, tag="xk", name=f"xq{qq}")
                    wk0 = wpool.tile([128, 2, U0], dt.float8e4, tag="wk", name=f"wk{2 * qq}")
                    wk1 = wpool.tile([128, 2, U0], dt.float8e4, tag="wk", name=f"wk{2 * qq + 1}")
                    r = slice(qq * 128, (qq + 1) * 128)
                    r0 = slice(2 * qq * 128, (2 * qq + 1) * 128)
                    r1 = slice((2 * qq + 1) * 128, (2 * qq + 2) * 128)
                    if qq == 0:
                        # ramp: interleave x j-slices + W k-halves across the
                        # two fast queues; first plain matmul starts after
                        # ~192KB instead of a full 512KB quad+pair
                        nc.gpsimd.dma_start(out=xk[:, 0, :], in_=xp[r, 0, :])
                        nc.scalar.dma_start(out=wk0[:, 0, :], in_=wp[r0, 0, :])
                        nc.scalar.dma_start(out=xk[:, 1, :], in_=xp[r, 1, :])
                        nc.gpsimd.dma_start(out=wk0[:, 1, :], in_=wp[r0, 1, :])
                        nc.gpsimd.dma_start(out=xk[:, 2:4, :], in_=xp[r, 2:4, :])
                        nc.scalar.dma_start(out=wk1[:], in_=wp[r1])
                    else:
                        # x on scalar; wk0 on gpsimd; wk1 mostly scalar
                        nc.scalar.dma_start(out=xk[:], in_=xp[r])
                        nc.gpsimd.dma_start(out=wk0[:], in_=wp[r0])
                        q = nc.scalar if qq % 4 == 3 else nc.gpsimd
                        q.dma_start(out=wk1[:], in_=wp[r1])
                    xts.append(xk)
                    wts.append((wk0, wk1))
                    # trickle w1 on sync behind the consts
                    if 2 <= qq and w1_next < 12:
                        j = w1_next
                        w1_next += 1
                        nc.sync.dma_start(
                            out=w1p_t[:, j * 4 : (j + 1) * 4, :],
                            in_=w1pd[:, j * 4 : (j + 1) * 4, :],
                        )
                    if qq == 14:
                        nc.sync.dma_start(out=w1e_t[:], in_=w1ed[:])
                for n in range(NCH):
                    for qq_ in range(qbsz):
                        qq = qstart + qq_
                        if qb == 0:
                            # first pair: plain per-chunk; second: DoubleRow
                            for j in range(2):
                                nc.tensor.matmul(
                                    ps_h[n][:],
                                    wts[qq_][0][:, j, n * 128 : (n + 1) * 128],
                                    xts[qq_][:, j, :],
                                    start=(qq == 0 and j == 0),
                                    stop=False,
                                )
                            nc.tensor.matmul(
                                ps_h[n][:],
                                wts[qq_][1][:, :, n * 128 : (n + 1) * 128],
                                xts[qq_][:, 2:4, :],
                                start=False,
                                stop=False,
                                perf_mode=DR,
                            )
                        else:
                            for hf in range(2):
                                nc.tensor.matmul(
                                    ps_h[n][:],
                                    wts[qq_][hf][:, :, n * 128 : (n + 1) * 128],
                                    xts[qq_][:, 2 * hf : 2 * hf + 2, :],
                                    start=False,
                                    stop=(qq == KQ - 1 and hf == 1),
                                    perf_mode=DR,
                                )
                    if last_block:
                        # n-chunk complete: drain to fp8 SBUF (relu + bias)
                        tgt = hp_pair[n // 2][:, n % 2, :]
                        if n % 2 == 0:
                            nc.scalar.activation(
                                tgt, ps_h[n][:], AF.Relu,
                                bias=bfc_t[:, n : n + 1], scale=1.0,
                            )
                        else:
                            nc.vector.tensor_scalar(
                                tgt, ps_h[n][:], bfc_t[:, n : n + 1], 0.0,
                                mybir.AluOpType.add, mybir.AluOpType.max,
                            )
                qstart += qbsz

            # ego rows + constant bias row land late (needed at head phase)
            nc.sync.dma_start(out=hpe[0 : EGO + 1, :], in_=egoT[:])

            # heads on column segments, masked accumulate into z2all
            z2all = const.tile([4, BC], dt.float32, tag="z2all")
            nc.vector.memset(z2all[:], 0.0)
            t4 = const.tile([4, BC], dt.float32, tag="t4")
            u4 = const.tile([4, BC], dt.float32, tag="u4")
            a4 = const.tile([4, BC], dt.float32, tag="a4")
            m4 = const.tile([4, BC], dt.float32, tag="m4")
            outt = const.tile([4, BC], dt.float32, tag="outt")
            # per-head one-hot masks on idle GpSimd
            mks = []
            for h in range(H):
                a, C = seg[h]
                mk = const.tile([4, C], dt.float32, tag=f"mk_{h}", name=f"mk_{h}")
                nc.vector.tensor_scalar(
                    mk[:], hid_t[:, a : a + C], float(h + 1), None,
                    mybir.AluOpType.is_equal,
                )
                mks.append(mk)

            fin = 0
            pending = []  # finalized-column slices, emitted one head late

            def _emit_epilogue(sl):
                # mean rows 0-1: 5*tanh(z/5); std rows 2-3:
                # softplus(z) ~= z + exp(-z) + 1e-4 (z ~ 5 here).
                # Per-partition epi constants make one formula cover both:
                # out = tanh(z*e0)*e2 + (z + exp(z*e1))*e3 + e4
                nc.scalar.activation(
                    u4[:, sl], z2all[:, sl], AF.Exp, scale=epi_t[:, 1:2]
                )
                nc.scalar.activation(
                    t4[:, sl], z2all[:, sl], AF.Tanh, scale=epi_t[:, 0:1]
                )
                nc.vector.tensor_add(a4[:, sl], z2all[:, sl], u4[:, sl])
                nc.vector.tensor_scalar(
                    a4[:, sl], a4[:, sl], epi_t[:, 3:4], epi_t[:, 4:5],
                    mybir.AluOpType.mult, mybir.AluOpType.add,
                )
                nc.vector.tensor_scalar(
                    m4[:, sl], t4[:, sl], epi_t[:, 2:3], None,
                    mybir.AluOpType.mult,
                )
                nc.vector.tensor_add(outt[:, sl], m4[:, sl], a4[:, sl])
                nc.scalar.dma_start(out=out_d[:, sl], in_=outt[:, sl])

            for h in range(H):
                a, C = seg[h]
                z1p = zpool.tile([128, 2, C], dt.float8e4, tag=f"z1_{h}")
                for m in range(MC):
                    pz = psum.tile([128, C], dt.float32, tag="ps", name=f"pz_{h}_{m}")
                    # ego+bias single chunk first (available earliest)
                    nc.tensor.matmul(
                        pz[:],
                        w1e_t[:, h * 256 + m * 128 : h * 256 + (m + 1) * 128],
                        hpe[:, a : a + C],
                        start=True,
                        stop=False,
                    )
                    for t in range(4):
                        g = (h * 4 + t) * 2
                        nc.tensor.matmul(
                            pz[:],
                            w1p_t[:, g : g + 2, m * 128 : (m + 1) * 128],
                            hp_pair[t][:, :, a : a + C],
                            start=False,
                            stop=(t == 3),
                            perf_mode=DR,
                        )
                    tgt = z1p[:, m, :]
                    if m == 0:
                        nc.scalar.activation(
                            tgt, pz[:], AF.Relu, scale=SZ / (SH * SW1)
                        )
                    else:
                        nc.vector.tensor_scalar(
                            tgt, pz[:], SZ / (SH * SW1), 0.0,
                            mybir.AluOpType.mult, mybir.AluOpType.max,
                        )
                pz2 = psum.tile([4, C], dt.float32, tag="ps", name=f"pz2_{h}")
                for m in range(MC):
                    nc.tensor.matmul(
                        pz2[:],
                        w2_t[:, h * 2 + m, :],
                        z1p[:, m, :],
                        start=(m == 0),
                        stop=(m == MC - 1),
                    )
                z2s = const.tile([4, C], dt.float32, tag=f"z2s_{h}", name=f"z2s_{h}")
                nc.scalar.activation(
                    z2s[:], pz2[:], AF.Identity,
                    bias=b2_t[:, h : h + 1], scale=1.0 / (SZ * SW2),
                )
                z2mh = const.tile([4, C], dt.float32, tag=f"z2m_{h}", name=f"z2m_{h}")
                nc.vector.tensor_mul(z2mh[:], z2s[:], mks[h][:])
                nc.vector.tensor_add(
                    z2all[:, a : a + C], z2all[:, a : a + C], z2mh[:]
                )

                while pending:
                    _emit_epilogue(pending.pop(0))
                end = seg[h + 1][0] if h < H - 1 else BC
                # batch ~2 heads per epilogue slice to cut op count
                if end > fin and (h % 2 == 1 or h == H - 1):
                    pending.append(slice(fin, end))
                    fin = end
            while pending:
                _emit_epilogue(pending.pop(0))

    nc.compile()
    return nc


def _route(command):
    """Deal samples to cores balanced per head; sort each core by head.

    Returns (perms, seg): perms[c] = global sample indices for core c in
    column order; seg[h] = (a_h, C_h) identical across cores, covering every
    head-h sample's column on every core.
    """
    command = np.asarray(command, dtype=np.int32)
    glob_counts = np.array([(command == h + 1).sum() for h in range(H)], np.int64)
    shares = np.tile(glob_counts // NCORES, (NCORES, 1))
    ptr = 0
    for h in range(H):
        for _ in range(int(glob_counts[h] % NCORES)):
            shares[ptr % NCORES, h] += 1
            ptr += 1
    assert (shares.sum(axis=1) == BC).all()
    percore = [[] for _ in range(NCORES)]
    counts = np.zeros((NCORES, H), np.int64)
    for h in range(H):
        idx = np.nonzero(command == h + 1)[0]
        off = 0
        for c in range(NCORES):
            share = idx[off : off + shares[c, h]]
            off += shares[c, h]
            percore[c].append(share)
            counts[c, h] = len(share)
    perms = [np.concatenate(percore[c]) for c in range(NCORES)]
    starts = np.zeros((NCORES, H), np.int64)
    starts[:, 1:] = np.cumsum(counts, axis=1)[:, :-1]
    ends = starts + counts
    seg = []
    for h in range(H):
        a = int(starts[:, h].min())
        e = int(ends[:, h].max())
        if e <= a:  # head empty on every core: 1 masked-off dummy column
            a = min(a, BC - 1)
            e = a + 1
        seg.append((a, e - a))
    for c in range(NCORES):
        for h in range(H):
            a, C = seg[h]
            assert starts[c, h] >= a and ends[c, h] <= a + C
        assert len(perms[c]) == BC
    return perms, seg


def _q8(a):
    return np.clip(a, -240.0, 240.0).astype(FP8)


def _prep_inputs(x, command, ego_state, W_fc, b_fc, W1, b1, W2, b2, perms):
    """Host-side shard + layout prep. Returns in_maps for 8 cores."""
    x = np.asarray(x, dtype=np.float32)
    command = np.asarray(command, dtype=np.int32)
    ego_state = np.asarray(ego_state, dtype=np.float32)

    xq = _q8(x)  # [B, EMBED] fp8
    # W_fc pairs: [kp*128+p, j, n] = 32*Wfc[kp*256 + j*128 + p, n]
    wp_host = np.ascontiguousarray(
        _q8(SH * np.asarray(W_fc, np.float32))
        .reshape(KP, 2, 128, U0)
        .transpose(0, 2, 1, 3)
        .reshape(KP * 128, 2, U0)
    )
    bfcT = np.ascontiguousarray(
        (SH * np.asarray(b_fc, np.float32)).reshape(NCH, 128).T
    )

    # W1 augmented: rows 0..1026 = W1, row 1027 = b1, pad to 1152
    W1 = np.asarray(W1, np.float32)
    w1a = np.zeros((H, 9 * 128, U1), np.float32)
    w1a[:, : U0 + EGO] = W1
    w1a[:, U0 + EGO] = np.asarray(b1, np.float32)
    w1q = _q8(SW1 * w1a)
    # pair part: [p, (h*4+t)*2+j, o] = w1q[h, (2t+j)*128 + p, o]
    w1p_host = np.ascontiguousarray(
        w1q[:, :U0].reshape(H, 4, 2, 128, U1).transpose(3, 0, 1, 2, 4)
        .reshape(128, H * 4 * 2, U1)
    )
    # ego chunk: [p, h*256 + o] = w1q[h, 1024 + p, o]
    w1e_host = np.ascontiguousarray(
        w1q[:, U0:].reshape(H, 128, U1).transpose(1, 0, 2).reshape(128, H * U1)
    )
    # W2 pairs: [p, h*2+j, d] = 16*W2[h, j*128 + p, d]
    w2p_host = np.ascontiguousarray(
        _q8(SW2 * np.asarray(W2, np.float32))
        .reshape(H, 2, 128, 4)
        .transpose(2, 0, 1, 3)
        .reshape(128, H * 2, 4)
    )
    raw_init_std = np.log(np.exp(INIT_STD) - 1.0).astype(np.float32)
    b2m = np.ascontiguousarray(
        np.asarray(b2, np.float32).T
        + np.array([0, 0, raw_init_std, raw_init_std], np.float32)[:, None]
    )  # [4, H]

    epi_host = np.array(
        [
            [1 / MEAN_SCALE, 0.0, MEAN_SCALE, 0.0, 0.0],
            [1 / MEAN_SCALE, 0.0, MEAN_SCALE, 0.0, 0.0],
            [0.0, -1.0, 0.0, 1.0, MIN_STD],
            [0.0, -1.0, 0.0, 1.0, MIN_STD],
        ],
        np.float32,
    )
    in_maps = []
    for c in range(NCORES):
        p = perms[c]
        # x pairs: [kp*128+q, j, b] = xq[perm[b], kp*256 + j*128 + q]
        xp_host = np.ascontiguousarray(
            xq[p].T.reshape(KQ, 4, 128, BC).transpose(0, 2, 1, 3)
            .reshape(KQ * 128, 4, BC)
        )
        cmd_c = command[p]
        hid_c = np.ascontiguousarray(
            np.broadcast_to(cmd_c[None, :].astype(np.float32), (4, BC)).copy()
        )
        in_maps.append(
            {
                "xp": xp_host,
                "epi": epi_host,
                "wp": wp_host,
                "bfcT": bfcT,
                "egoT": np.ascontiguousarray(
                    _q8(np.concatenate(
                        [SH * ego_state[p].T, np.full((1, BC), SH, np.float32)], 0
                    ))
                ),
                "w1pd": w1p_host,
                "w1ed": w1e_host,
                "w2pd": w2p_host,
                "b2m": b2m,
                "hid": hid_c,
            }
        )
    return in_maps


def run(inputs, trace=False):
    """Build, run on 8 cores; returns (full output [4096,4] f32, results)."""
    from concourse.bass_utils import run_bass_kernel_spmd

    perms, seg = _route(inputs["command"])
    in_maps = _prep_inputs(**inputs, perms=perms)
    nc = _build_graph(seg)
    res = run_bass_kernel_spmd(nc, in_maps, core_ids=list(range(NCORES)), trace=trace)
    full = np.empty((B, 4), np.float32)
    for c in range(NCORES):
        full[perms[c]] = res.results[c]["out"].T
    return full, res


def kernel(**inputs):
    out, _ = run(inputs, trace=False)
    return out


# revision 18
# speedup vs baseline: 1.0711x; 1.0711x over previous
"""Trainium2 Bass kernel for nn_ActionDecoder (moe_routing) — fp8 DoubleRow.

Data-parallel across 8 NeuronCores: batch 4096 -> 512 per core, weights
replicated. Host deals samples to cores balanced per command value and sorts
each core's 512 samples by command, so each head's samples occupy a fixed
column segment [a_h, e_h) (identical offsets on all cores -> one SPMD graph).

All GEMMs run in fp8-e4m3 DoubleRow perf mode (2 k-rows per cycle, 157 TF/s)
with f32 PSUM accumulation. Power-of-two scales keep quantization exact to
descale: x as-is, W_fc*32 -> hp stores 32*h' (range <= 178 < 240 fp8 max),
W1*32 with b1 riding an augmented constant row, z1 stores 16*relu(.),
W2*16 -> z2 = psum/256 + b2.

Per core:
  hp  = relu((x@32Wfc) + 32 b_fc)             [128x8 x 512] fp8, 4 pair tiles
  per head h on columns [a_h, e_h):
    z1 = relu(psum/64)                        [256, C] -> [128,2,C] fp8
    z2 = W2'.T z1 / 256 + b2'                 [4, C]
    z2all[:, seg] += z2 * onehot(command)
  out = [5*tanh(mean/5); softplus(std)]       [4, 512] f32
"""

import numpy as np
import ml_dtypes

B = 4096
EMBED = 8192
U0 = 1024
U1 = 256
EGO = 3
H = 6
NCORES = 8
BC = B // NCORES          # 512 batch per core
KP = EMBED // 256         # 32 trunk k-pairs (DoubleRow: 256 k per matmul)
KO = EMBED // 1024        # 8 trunk k-octs (one x DMA each, 4KB lines)
PB = 4                    # trunk pairs per block
NCH = U0 // 128           # 8 trunk n-chunks
MC = U1 // 128            # 2 head m-chunks
NWARM = 64                # junk matmuls to warm the PE clock during DMA ramp
MEAN_SCALE = 5.0
INIT_STD = 5.0
MIN_STD = 1e-4
SH = 32.0                 # scale on W_fc / hp
SW1 = 32.0                # scale on W1 (+bias row)
SZ = 16.0                 # scale on stored z1
SW2 = 16.0                # scale on W2

FP8 = ml_dtypes.float8_e4m3   # TRN fp8_e4m3 semantics (max 240)


def _build_graph(seg):
    """seg: list of (a_h, C_h) column segments per head, identical on all cores."""
    import concourse.mybir as mybir
    import concourse.tile as tile
    from concourse import bacc

    dt = mybir.dt
    AF = mybir.ActivationFunctionType
    DR = mybir.MatmulPerfMode.DoubleRow

    nc = bacc.Bacc("TRN2", target_bir_lowering=False, debug=False)

    xp = nc.dram_tensor("xp", [KO * 128, 8, BC], dt.float8e4, kind="ExternalInput")
    wp = nc.dram_tensor("wp", [KP * 64, 4, U0], dt.float8e4, kind="ExternalInput")
    bfcT = nc.dram_tensor("bfcT", [128, NCH], dt.float32, kind="ExternalInput")
    egoT = nc.dram_tensor("egoT", [EGO + 1, BC], dt.float8e4, kind="ExternalInput")
    w1pd = nc.dram_tensor("w1pd", [128, H * 4 * 2, 256], dt.float8e4, kind="ExternalInput")
    w1ed = nc.dram_tensor("w1ed", [128, H * 256], dt.float8e4, kind="ExternalInput")
    w2pd = nc.dram_tensor("w2pd", [128, H * 2, 4], dt.float8e4, kind="ExternalInput")
    b2m = nc.dram_tensor("b2m", [4, H], dt.float32, kind="ExternalInput")
    hid = nc.dram_tensor("hid", [4, BC], dt.float32, kind="ExternalInput")
    epi = nc.dram_tensor("epi", [4, 5], dt.float32, kind="ExternalInput")
    out_d = nc.dram_tensor("out", [4, BC], dt.float32, kind="ExternalOutput")

    with tile.TileContext(nc) as tc:
        with (
            tc.tile_pool(name="const", bufs=1) as const,
            tc.tile_pool(name="xk", bufs=8) as xpool,
            tc.tile_pool(name="wk", bufs=16) as wpool,
            tc.tile_pool(name="hp", bufs=1) as hpool,
            tc.tile_pool(name="z1", bufs=1) as zpool,
            tc.tile_pool(name="ps", bufs=8, space="PSUM") as psum,
        ):
            # constants / small inputs on the slow sync queue (~2MB total
            # incl. w1): the fast scalar/gpsimd queues carry only x and W.
            bfc_t = const.tile([128, NCH], dt.float32, tag="bfc")
            nc.sync.dma_start(out=bfc_t[:], in_=bfcT[:])
            b2_t = const.tile([4, H], dt.float32, tag="b2")
            nc.sync.dma_start(out=b2_t[:], in_=b2m[:])
            hid_t = const.tile([4, BC], dt.float32, tag="hid")
            nc.sync.dma_start(out=hid_t[:], in_=hid[:])
            w2_t = const.tile([128, H * 2, 4], dt.float8e4, tag="w2")
            nc.sync.dma_start(out=w2_t[:], in_=w2pd[:])
            epi_t = const.tile([4, 5], dt.float32, tag="epi")
            nc.sync.dma_start(out=epi_t[:], in_=epi[:])
            w1p_t = const.tile([128, H * 4 * 2, 256], dt.float8e4, tag="w1p")
            w1e_t = const.tile([128, H * 256], dt.float8e4, tag="w1e")

            # h' ego chunk: rows 0-2 = 32*ego, row 3 = 32 (bias row), rest 0
            hpe = hpool.tile([128, BC], dt.float8e4, tag="hpe")
            nc.vector.memset(hpe[:], 0.0)

            ps_h = [
                psum.tile([128, BC], dt.float32, tag="ps", name=f"ps_h{n}")
                for n in range(NCH)
            ]

            # PE warmup: junk matmuls on the zeroed tile keep the PE activity
            # monitor busy while the first trunk chunks stream in, so the
            # trunk starts at 2.4 GHz instead of 1.2.
            for i in range(NWARM):
                nc.tensor.matmul(
                    ps_h[NCH - 1][:, 0:64],
                    hpe[:, 0:128],
                    hpe[:, 128:192],
                    start=True,
                    stop=True,
                )

            hp_pair = [
                hpool.tile([128, 2, BC], dt.float8e4, tag=f"hpp{t}", name=f"hpp{t}")
                for t in range(4)
            ]

            # trunk: hp = relu((32Wfc).T @ x + 32 b_fc), DoubleRow over
            # k-pairs; x streams in k-octs and W in k-quads (512KB DMAs with
            # 4KB partition lines — DMA queues are packet-rate-bound, so
            # bigger lines mean proportionally more bandwidth).
            w1_next = 0
            for ob in range(KO):
                first_block = ob == 0
                last_block = ob == KO - 1
                xk = xpool.tile([128, 8, BC], dt.float8e4, tag="xk", name=f"xo{ob}")
                wq0 = wpool.tile([128, 4, U0], dt.float8e4, tag="wk", name=f"wq{2 * ob}")
                wq1 = wpool.tile([128, 4, U0], dt.float8e4, tag="wk", name=f"wq{2 * ob + 1}")
                r = slice(ob * 128, (ob + 1) * 128)
                r0 = slice(2 * ob * 128, (2 * ob + 1) * 128)
                r1 = slice((2 * ob + 1) * 128, (2 * ob + 2) * 128)
                if first_block:
                    # ramp: j-sliced x + pair-sliced W interleaved across the
                    # two fast queues; first plain matmul starts after ~192KB
                    nc.gpsimd.dma_start(out=xk[:, 0, :], in_=xp[r, 0, :])
                    nc.scalar.dma_start(out=wq0[:, 0:2, 0:512], in_=wp[r0, 0:2, 0:512])
                    nc.scalar.dma_start(out=xk[:, 1, :], in_=xp[r, 1, :])
                    nc.gpsimd.dma_start(out=wq0[:, 0:2, 512:], in_=wp[r0, 0:2, 512:])
                    nc.gpsimd.dma_start(out=xk[:, 2:4, :], in_=xp[r, 2:4, :])
                    nc.scalar.dma_start(out=wq0[:, 2:4, :], in_=wp[r0, 2:4, :])
                    nc.scalar.dma_start(out=xk[:, 4:8, :], in_=xp[r, 4:8, :])
                    nc.gpsimd.dma_start(out=wq1[:], in_=wp[r1])
                else:
                    # alternate queues per oct: x and its two w-quads split
                    qa, qb_ = (nc.scalar, nc.gpsimd) if ob % 2 else (nc.gpsimd, nc.scalar)
                    qa.dma_start(out=xk[:], in_=xp[r])
                    qb_.dma_start(out=wq0[:], in_=wp[r0])
                    qa.dma_start(out=wq1[:], in_=wp[r1])
                # trickle w1 on sync behind the consts
                while w1_next < 12 and w1_next <= 2 * (ob - 1):
                    j = w1_next
                    w1_next += 1
                    nc.sync.dma_start(
                        out=w1p_t[:, j * 4 : (j + 1) * 4, :],
                        in_=w1pd[:, j * 4 : (j + 1) * 4, :],
                    )
                if ob == KO - 2:
                    nc.sync.dma_start(out=w1e_t[:], in_=w1ed[:])
                for n in range(NCH):
                    if first_block:
                        # first pair: plain per-chunk; rest: DoubleRow
                        for j in range(2):
                            nc.tensor.matmul(
                                ps_h[n][:],
                                wq0[:, j, n * 128 : (n + 1) * 128],
                                xk[:, j, :],
                                start=(j == 0),
                                stop=False,
                            )
                        for pp in range(1, 4):
                            wq, u = (wq0, 1) if pp < 2 else (wq1, pp - 2)
                            nc.tensor.matmul(
                                ps_h[n][:],
                                wq[:, 2 * u : 2 * u + 2, n * 128 : (n + 1) * 128],
                                xk[:, 2 * pp : 2 * pp + 2, :],
                                start=False,
                                stop=False,
                                perf_mode=DR,
                            )
                    else:
                        for pp in range(4):
                            wq, u = (wq0, pp) if pp < 2 else (wq1, pp - 2)
                            nc.tensor.matmul(
                                ps_h[n][:],
                                wq[:, 2 * u : 2 * u + 2, n * 128 : (n + 1) * 128],
                                xk[:, 2 * pp : 2 * pp + 2, :],
                                start=False,
                                stop=(last_block and pp == 3),
                                perf_mode=DR,
                            )
                    if last_block:
                        # n-chunk complete: drain to fp8 SBUF (relu + bias)
                        tgt = hp_pair[n // 2][:, n % 2, :]
                        if n % 2 == 0:
                            nc.scalar.activation(
                                tgt, ps_h[n][:], AF.Relu,
                                bias=bfc_t[:, n : n + 1], scale=1.0,
                            )
                        else:
                            nc.vector.tensor_scalar(
                                tgt, ps_h[n][:], bfc_t[:, n : n + 1], 0.0,
                                mybir.AluOpType.add, mybir.AluOpType.max,
                            )

            # ego rows + constant bias row land late (needed at head phase)
            nc.sync.dma_start(out=hpe[0 : EGO + 1, :], in_=egoT[:])

            # heads on column segments, masked accumulate into z2all
            z2all = const.tile([4, BC], dt.float32, tag="z2all")
            nc.vector.memset(z2all[:], 0.0)
            t4 = const.tile([4, BC], dt.float32, tag="t4")
            u4 = const.tile([4, BC], dt.float32, tag="u4")
            a4 = const.tile([4, BC], dt.float32, tag="a4")
            m4 = const.tile([4, BC], dt.float32, tag="m4")
            outt = const.tile([4, BC], dt.float32, tag="outt")
            # per-head one-hot masks on idle GpSimd
            mks = []
            for h in range(H):
                a, C = seg[h]
                mk = const.tile([4, C], dt.float32, tag=f"mk_{h}", name=f"mk_{h}")
                nc.vector.tensor_scalar(
                    mk[:], hid_t[:, a : a + C], float(h + 1), None,
                    mybir.AluOpType.is_equal,
                )
                mks.append(mk)

            fin = 0
            pending = []  # finalized-column slices, emitted one head late

            def _emit_epilogue(sl):
                # mean rows 0-1: 5*tanh(z/5); std rows 2-3:
                # softplus(z) ~= z + exp(-z) + 1e-4 (z ~ 5 here).
                # Per-partition epi constants make one formula cover both:
                # out = tanh(z*e0)*e2 + (z + exp(z*e1))*e3 + e4
                nc.scalar.activation(
                    u4[:, sl], z2all[:, sl], AF.Exp, scale=epi_t[:, 1:2]
                )
                nc.scalar.activation(
                    t4[:, sl], z2all[:, sl], AF.Tanh, scale=epi_t[:, 0:1]
                )
                nc.vector.tensor_add(a4[:, sl], z2all[:, sl], u4[:, sl])
                nc.vector.tensor_scalar(
                    a4[:, sl], a4[:, sl], epi_t[:, 3:4], epi_t[:, 4:5],
                    mybir.AluOpType.mult, mybir.AluOpType.add,
                )
                nc.vector.tensor_scalar(
                    m4[:, sl], t4[:, sl], epi_t[:, 2:3], None,
                    mybir.AluOpType.mult,
                )
                nc.vector.tensor_add(outt[:, sl], m4[:, sl], a4[:, sl])
                nc.scalar.dma_start(out=out_d[:, sl], in_=outt[:, sl])

            for h in range(H):
                a, C = seg[h]
                z1p = zpool.tile([128, 2, C], dt.float8e4, tag=f"z1_{h}")
                for m in range(MC):
                    pz = psum.tile([128, C], dt.float32, tag="ps", name=f"pz_{h}_{m}")
                    # ego+bias single chunk first (available earliest)
                    nc.tensor.matmul(
                        pz[:],
                        w1e_t[:, h * 256 + m * 128 : h * 256 + (m + 1) * 128],
                        hpe[:, a : a + C],
                        start=True,
                        stop=False,
                    )
                    for t in range(4):
                        g = (h * 4 + t) * 2
                        nc.tensor.matmul(
                            pz[:],
                            w1p_t[:, g : g + 2, m * 128 : (m + 1) * 128],
                            hp_pair[t][:, :, a : a + C],
                            start=False,
                            stop=(t == 3),
                            perf_mode=DR,
                        )
                    tgt = z1p[:, m, :]
                    if m == 0:
                        nc.scalar.activation(
                            tgt, pz[:], AF.Relu, scale=SZ / (SH * SW1)
                        )
                    else:
                        nc.vector.tensor_scalar(
                            tgt, pz[:], SZ / (SH * SW1), 0.0,
                            mybir.AluOpType.mult, mybir.AluOpType.max,
                        )
                pz2 = psum.tile([4, C], dt.float32, tag="ps", name=f"pz2_{h}")
                for m in range(MC):
                    nc.tensor.matmul(
                        pz2[:],
                        w2_t[:, h * 2 + m, :],
                        z1p[:, m, :],
                        start=(m == 0),
                        stop=(m == MC - 1),
                    )
                z2s = const.tile([4, C], dt.float32, tag=f"z2s_{h}", name=f"z2s_{h}")
                nc.scalar.activation(
                    z2s[:], pz2[:], AF.Identity,
                    bias=b2_t[:, h : h + 1], scale=1.0 / (SZ * SW2),
                )
                z2mh = const.tile([4, C], dt.float32, tag=f"z2m_{h}", name=f"z2m_{h}")
                nc.vector.tensor_mul(z2mh[:], z2s[:], mks[h][:])
                nc.vector.tensor_add(
                    z2all[:, a : a + C], z2all[:, a : a + C], z2mh[:]
                )

                while pending:
                    _emit_epilogue(pending.pop(0))
                end = seg[h + 1][0] if h < H - 1 else BC
                # batch ~2 heads per epilogue slice to cut op count
                if end > fin and (h % 2 == 1 or h == H - 1):
                    pending.append(slice(fin, end))
                    fin = end
            while pending:
                _emit_epilogue(pending.pop(0))

    nc.compile()
    return nc


def _route(command):
    """Deal samples to cores balanced per head; sort each core by head.

    Returns (perms, seg): perms[c] = global sample indices for core c in
    column order; seg[h] = (a_h, C_h) identical across cores, covering every
    head-h sample's column on every core.
    """
    command = np.asarray(command, dtype=np.int32)
    glob_counts = np.array([(command == h + 1).sum() for h in range(H)], np.int64)
    shares = np.tile(glob_counts // NCORES, (NCORES, 1))
    ptr = 0
    for h in range(H):
        for _ in range(int(glob_counts[h] % NCORES)):
            shares[ptr % NCORES, h] += 1
            ptr += 1
    assert (shares.sum(axis=1) == BC).all()
    percore = [[] for _ in range(NCORES)]
    counts = np.zeros((NCORES, H), np.int64)
    for h in range(H):
        idx = np.nonzero(command == h + 1)[0]
        off = 0
        for c in range(NCORES):
            share = idx[off : off + shares[c, h]]
            off += shares[c, h]
            percore[c].append(share)
            counts[c, h] = len(share)
    perms = [np.concatenate(percore[c]) for c in range(NCORES)]
    starts = np.zeros((NCORES, H), np.int64)
    starts[:, 1:] = np.cumsum(counts, axis=1)[:, :-1]
    ends = starts + counts
    seg = []
    for h in range(H):
        a = int(starts[:, h].min())
        e = int(ends[:, h].max())
        if e <= a:  # head empty on every core: 1 masked-off dummy column
            a = min(a, BC - 1)
            e = a + 1
        seg.append((a, e - a))
    for c in range(NCORES):
        for h in range(H):
            a, C = seg[h]
            assert starts[c, h] >= a and ends[c, h] <= a + C
        assert len(perms[c]) == BC
    return perms, seg


def _q8(a):
    return np.clip(a, -240.0, 240.0).astype(FP8)


def _prep_inputs(x, command, ego_state, W_fc, b_fc, W1, b1, W2, b2, perms):
    """Host-side shard + layout prep. Returns in_maps for 8 cores."""
    x = np.asarray(x, dtype=np.float32)
    command = np.asarray(command, dtype=np.int32)
    ego_state = np.asarray(ego_state, dtype=np.float32)

    xq = _q8(x)  # [B, EMBED] fp8
    # W_fc pairs: [kp*128+p, j, n] = 32*Wfc[kp*256 + j*128 + p, n]
    wp_host = np.ascontiguousarray(
        _q8(SH * np.asarray(W_fc, np.float32))
        .reshape(KP // 2, 4, 128, U0)
        .transpose(0, 2, 1, 3)
        .reshape(KP * 64, 4, U0)
    )
    bfcT = np.ascontiguousarray(
        (SH * np.asarray(b_fc, np.float32)).reshape(NCH, 128).T
    )

    # W1 augmented: rows 0..1026 = W1, row 1027 = b1, pad to 1152
    W1 = np.asarray(W1, np.float32)
    w1a = np.zeros((H, 9 * 128, U1), np.float32)
    w1a[:, : U0 + EGO] = W1
    w1a[:, U0 + EGO] = np.asarray(b1, np.float32)
    w1q = _q8(SW1 * w1a)
    # pair part: [p, (h*4+t)*2+j, o] = w1q[h, (2t+j)*128 + p, o]
    w1p_host = np.ascontiguousarray(
        w1q[:, :U0].reshape(H, 4, 2, 128, U1).transpose(3, 0, 1, 2, 4)
        .reshape(128, H * 4 * 2, U1)
    )
    # ego chunk: [p, h*256 + o] = w1q[h, 1024 + p, o]
    w1e_host = np.ascontiguousarray(
        w1q[:, U0:].reshape(H, 128, U1).transpose(1, 0, 2).reshape(128, H * U1)
    )
    # W2 pairs: [p, h*2+j, d] = 16*W2[h, j*128 + p, d]
    w2p_host = np.ascontiguousarray(
        _q8(SW2 * np.asarray(W2, np.float32))
        .reshape(H, 2, 128, 4)
        .transpose(2, 0, 1, 3)
        .reshape(128, H * 2, 4)
    )
    raw_init_std = np.log(np.exp(INIT_STD) - 1.0).astype(np.float32)
    b2m = np.ascontiguousarray(
        np.asarray(b2, np.float32).T
        + np.array([0, 0, raw_init_std, raw_init_std], np.float32)[:, None]
    )  # [4, H]

    epi_host = np.array(
        [
            [1 / MEAN_SCALE, 0.0, MEAN_SCALE, 0.0, 0.0],
            [1 / MEAN_SCALE, 0.0, MEAN_SCALE, 0.0, 0.0],
            [0.0, -1.0, 0.0, 1.0, MIN_STD],
            [0.0, -1.0, 0.0, 1.0, MIN_STD],
        ],
        np.float32,
    )
    in_maps = []
    for c in range(NCORES):
        p = perms[c]
        # x pairs: [kp*128+q, j, b] = xq[perm[b], kp*256 + j*128 + q]
        xp_host = np.ascontiguousarray(
            xq[p].T.reshape(KO, 8, 128, BC).transpose(0, 2, 1, 3)
            .reshape(KO * 128, 8, BC)
        )
        cmd_c = command[p]
        hid_c = np.ascontiguousarray(
            np.broadcast_to(cmd_c[None, :].astype(np.float32), (4, BC)).copy()
        )
        in_maps.append(
            {
                "xp": xp_host,
                "epi": epi_host,
                "wp": wp_host,
                "bfcT": bfcT,
                "egoT": np.ascontiguousarray(
                    _q8(np.concatenate(
                        [SH * ego_state[p].T, np.full((1, BC), SH, np.float32)], 0
                    ))
                ),
                "w1pd": w1p_host,
                "w1ed": w1e_host,
                "w2pd": w2p_host,
                "b2m": b2m,
                "hid": hid_c,
            }
        )
    return in_maps


def run(inputs, trace=False):
    """Build, run on 8 cores; returns (full output [4096,4] f32, results)."""
    from concourse.bass_utils import run_bass_kernel_spmd

    perms, seg = _route(inputs["command"])
    in_maps = _prep_inputs(**inputs, perms=perms)
    nc = _build_graph(seg)
    res = run_bass_kernel_spmd(nc, in_maps, core_ids=list(range(NCORES)), trace=trace)
    full = np.empty((B, 4), np.float32)
    for c in range(NCORES):
        full[perms[c]] = res.results[c]["out"].T
    return full, res


def kernel(**inputs):
    out, _ = run(inputs, trace=False)
    return out
